# revision 2
# baseline (speedup 1.0000x reference)
"""3x3 SAME conv (B=32, Cin=128, H=W=64, Cout=256) + bias + relu on 8 trn2 cores.

Strategy: data-parallel over batch (4 images per core, no collectives).
Per image, implicit GEMM: input held in SBUF as a zero-padded [Cin=128, 66, 66]
f32r tile; for each of 9 taps a [128cin x 128cout] weight slice multiplies a
shifted [128, 8rows*64cols] window, accumulating in PSUM. float32r (1-pass
reduced-precision fp32 matmul) runs at full PE rate with ~1e-4 rel err.
Bias+relu fused on the scalar engine straight out of PSUM.
"""

from contextlib import ExitStack

import numpy as np

import concourse.bass as bass
import concourse.tile as tile
from concourse import bacc, mybir
from concourse.bass_utils import run_bass_kernel_spmd

N_CORES = 8
B, C_IN, H, W = 32, 128, 64, 64
C_OUT, K = 256, 3
B_LOC = B // N_CORES          # images per core
N_CHUNK = C_OUT // 128        # cout chunks of 128
ROWS_PER_MM = 8               # 8 rows x 64 cols = 512 moving elements
N_RG = H // ROWS_PER_MM       # row groups per image
HP, WP = H + 2, W + 2         # padded

_COMPILED = None


def _build():
    nc = bacc.Bacc("TRN2", target_bir_lowering=False, debug=False,
                   num_devices=N_CORES)

    inp = nc.dram_tensor("inp", [B_LOC, C_IN, H, W], mybir.dt.float32,
                         kind="ExternalInput").ap()
    wt = nc.dram_tensor("wt", [C_IN, K * K, C_OUT], mybir.dt.float32,
                        kind="ExternalInput").ap()
    bias2 = nc.dram_tensor("bias2", [128, N_CHUNK], mybir.dt.float32,
                           kind="ExternalInput").ap()
    out = nc.dram_tensor("out", [B_LOC, C_OUT, H, W], mybir.dt.float32,
                         kind="ExternalOutput").ap()

    with tile.TileContext(nc) as tc, ExitStack() as ctx:
        consts = ctx.enter_context(tc.tile_pool(name="consts", bufs=1))
        pads = ctx.enter_context(tc.tile_pool(name="pads", bufs=1))
        raws = ctx.enter_context(tc.tile_pool(name="raws", bufs=3))
        outs = ctx.enter_context(tc.tile_pool(name="outs", bufs=4))
        psums = ctx.enter_context(tc.tile_pool(name="psums", bufs=4,
                                               space="PSUM"))

        w_f32 = consts.tile([128, K * K, C_OUT], mybir.dt.float32, tag="w_f32")
        nc.sync.dma_start(out=w_f32[:], in_=wt[:])
        w_r = consts.tile([128, K * K, C_OUT], mybir.dt.float32r, tag="w_r")
        nc.vector.tensor_copy(w_r[:], w_f32[:])

        b_sb = consts.tile([128, N_CHUNK], mybir.dt.float32, tag="b_sb")
        nc.sync.dma_start(out=b_sb[:], in_=bias2[:])

        # Padded f32r image buffers; borders zeroed once (via an f32 scratch
        # rounded through the DVE, since memset can't emit f32r), interior
        # rewritten per image.
        NBUF = 3
        zscratch = consts.tile([128, HP, WP], mybir.dt.float32, tag="zscratch")
        nc.vector.memset(zscratch[:], 0.0)
        pimgs = []
        for i in range(NBUF):
            p = pads.tile([128, HP, WP], mybir.dt.float32r, tag=f"pimg{i}")
            nc.vector.tensor_copy(p[:], zscratch[:])
            pimgs.append(p)

        for b in range(B_LOC):
            raw = raws.tile([128, H, W], mybir.dt.float32, tag="raw")
            nc.sync.dma_start(out=raw[:], in_=inp[b])
            pimg = pimgs[b % NBUF]
            nc.vector.tensor_copy(pimg[:, 1:H + 1, 1:W + 1], raw[:])

            for c in range(N_CHUNK):
                for r in range(N_RG):
                    acc = psums.tile([128, ROWS_PER_MM * W], mybir.dt.float32,
                                     tag="acc")
                    y0 = r * ROWS_PER_MM
                    for t in range(K * K):
                        kh, kw = divmod(t, K)
                        rhs = pimg[:, y0 + kh:y0 + kh + ROWS_PER_MM,
                                   kw:kw + W]
                        nc.tensor.matmul(acc[:],
                                         w_r[:, t, c * 128:(c + 1) * 128],
                                         rhs,
                                         start=(t == 0), stop=(t == K * K - 1))
                    o = outs.tile([128, ROWS_PER_MM, W], mybir.dt.float32,
                                  tag="o")
                    nc.scalar.activation(o[:], acc[:].rearrange(
                        "p (h w) -> p h w", h=ROWS_PER_MM),
                        mybir.ActivationFunctionType.Relu,
                        bias=b_sb[:, c:c + 1], scale=1.0)
                    nc.sync.dma_start(
                        out=out[b, c * 128:(c + 1) * 128,
                                y0:y0 + ROWS_PER_MM, :],
                        in_=o[:])

    nc.compile()
    return nc


def _get_compiled():
    global _COMPILED
    if _COMPILED is None:
        _COMPILED = _build()
    return _COMPILED


def _run(inp, weight, bias, trace=False):
    inp = np.ascontiguousarray(np.asarray(inp, dtype=np.float32))
    weight = np.asarray(weight, dtype=np.float32)
    bias = np.asarray(bias, dtype=np.float32)

    # weight [C_OUT, C_IN*K*K] -> [C_IN, K*K, C_OUT] (lhsT layout per tap)
    wt = np.ascontiguousarray(
        weight.reshape(C_OUT, C_IN, K * K).transpose(1, 2, 0))
    # bias [C_OUT] -> [128, N_CHUNK]: bias2[p, c] = bias[c*128 + p]
    bias2 = np.ascontiguousarray(bias.reshape(N_CHUNK, 128).T)

    nc = _get_compiled()
    in_maps = [
        {"inp": inp[i * B_LOC:(i + 1) * B_LOC], "wt": wt, "bias2": bias2}
        for i in range(N_CORES)
    ]
    res = run_bass_kernel_spmd(nc, in_maps, list(range(N_CORES)), trace=trace)
    full = np.concatenate([res.results[i]["out"] for i in range(N_CORES)],
                          axis=0)
    return full, res


def kernel(inp, weight, bias):
    full, _ = _run(inp, weight, bias, trace=False)
    return full


# revision 4
# speedup vs baseline: 1.0509x; 1.0509x over previous
"""3x3 SAME conv (B=32, Cin=128, H=W=64, Cout=256) + bias + relu on 8 trn2 cores.

Strategy: data-parallel over batch (4 images per core, no collectives).
Per image, implicit GEMM: input held in SBUF as a zero-padded [Cin=128, 66, 66]
f32r tile; for each of 9 taps a [128cin x 128cout] weight slice multiplies a
shifted [128, 8rows*64cols] window, accumulating in PSUM. float32r (1-pass
reduced-precision fp32 matmul) runs at full PE rate with ~1e-4 rel err.
Bias+relu fused on the scalar engine straight out of PSUM.
"""

from contextlib import ExitStack

import numpy as np

import concourse.bass as bass
import concourse.tile as tile
from concourse import bacc, mybir
from concourse.bass_utils import run_bass_kernel_spmd

N_CORES = 8
B, C_IN, H, W = 32, 128, 64, 64
C_OUT, K = 256, 3
B_LOC = B // N_CORES          # images per core
N_CHUNK = C_OUT // 128        # cout chunks of 128
ROWS_PER_MM = 8               # 8 rows x 64 cols = 512 moving elements
N_RG = H // ROWS_PER_MM       # row groups per image
HP, WP = H + 2, W + 2         # padded

_COMPILED = None


def _build():
    nc = bacc.Bacc("TRN2", target_bir_lowering=False, debug=False,
                   num_devices=N_CORES)

    inp = nc.dram_tensor("inp", [B_LOC, C_IN, H, W], mybir.dt.float32,
                         kind="ExternalInput").ap()
    wt = nc.dram_tensor("wt", [C_IN, K * K, C_OUT], mybir.dt.float32,
                        kind="ExternalInput").ap()
    bias2 = nc.dram_tensor("bias2", [128, N_CHUNK], mybir.dt.float32,
                           kind="ExternalInput").ap()
    out = nc.dram_tensor("out", [B_LOC, C_OUT, H, W], mybir.dt.float32,
                         kind="ExternalOutput").ap()

    with tile.TileContext(nc) as tc, ExitStack() as ctx:
        consts = ctx.enter_context(tc.tile_pool(name="consts", bufs=1))
        pads = ctx.enter_context(tc.tile_pool(name="pads", bufs=1))
        raws = ctx.enter_context(tc.tile_pool(name="raws", bufs=6))
        outs = ctx.enter_context(tc.tile_pool(name="outs", bufs=6))
        psums = ctx.enter_context(tc.tile_pool(name="psums", bufs=6,
                                               space="PSUM"))

        w_f32 = consts.tile([128, K * K, C_OUT], mybir.dt.float32, tag="w_f32")
        nc.sync.dma_start(out=w_f32[:], in_=wt[:])
        w_r = consts.tile([128, K * K, C_OUT], mybir.dt.float32r, tag="w_r")
        nc.vector.tensor_copy(w_r[:], w_f32[:])

        b_sb = consts.tile([128, N_CHUNK], mybir.dt.float32, tag="b_sb")
        nc.sync.dma_start(out=b_sb[:], in_=bias2[:])

        # Padded f32r image buffers; border strips zeroed once (via a small
        # f32 zero scratch rounded through the DVE, since memset can't emit
        # f32r), interior rewritten per image.
        NBUF = 3
        zscratch = consts.tile([128, HP], mybir.dt.float32, tag="zscratch")
        nc.vector.memset(zscratch[:], 0.0)
        pimgs = []
        for i in range(NBUF):
            p = pads.tile([128, HP, WP], mybir.dt.float32r, tag=f"pimg{i}")
            nc.vector.tensor_copy(p[:, 0, :], zscratch[:, :WP])
            nc.vector.tensor_copy(p[:, HP - 1, :], zscratch[:, :WP])
            nc.vector.tensor_copy(p[:, :, 0], zscratch[:])
            nc.vector.tensor_copy(p[:, :, WP - 1], zscratch[:])
            pimgs.append(p)

        # Load + round each image in horizontal bands so the first row
        # groups' matmuls can start before the whole image has landed.
        NBAND = 4
        BROWS = H // NBAND
        for b in range(B_LOC):
            pimg = pimgs[b % NBUF]
            for s in range(NBAND):
                raw = raws.tile([128, BROWS, W], mybir.dt.float32, tag="raw")
                nc.sync.dma_start(out=raw[:],
                                  in_=inp[b, :, s * BROWS:(s + 1) * BROWS, :])
                nc.vector.tensor_copy(
                    pimg[:, 1 + s * BROWS:1 + (s + 1) * BROWS, 1:W + 1],
                    raw[:])

            for c in range(N_CHUNK):
                for r in range(N_RG):
                    acc = psums.tile([128, ROWS_PER_MM * W], mybir.dt.float32,
                                     tag="acc")
                    y0 = r * ROWS_PER_MM
                    for t in range(K * K):
                        kh, kw = divmod(t, K)
                        rhs = pimg[:, y0 + kh:y0 + kh + ROWS_PER_MM,
                                   kw:kw + W]
                        nc.tensor.matmul(acc[:],
                                         w_r[:, t, c * 128:(c + 1) * 128],
                                         rhs,
                                         start=(t == 0), stop=(t == K * K - 1))
                    o = outs.tile([128, ROWS_PER_MM, W], mybir.dt.float32,
                                  tag="o")
                    nc.scalar.activation(o[:], acc[:].rearrange(
                        "p (h w) -> p h w", h=ROWS_PER_MM),
                        mybir.ActivationFunctionType.Relu,
                        bias=b_sb[:, c:c + 1], scale=1.0)
                    nc.sync.dma_start(
                        out=out[b, c * 128:(c + 1) * 128,
                                y0:y0 + ROWS_PER_MM, :],
                        in_=o[:])

    nc.compile()
    return nc


def _get_compiled():
    global _COMPILED
    if _COMPILED is None:
        _COMPILED = _build()
    return _COMPILED


def _run(inp, weight, bias, trace=False):
    inp = np.ascontiguousarray(np.asarray(inp, dtype=np.float32))
    weight = np.asarray(weight, dtype=np.float32)
    bias = np.asarray(bias, dtype=np.float32)

    # weight [C_OUT, C_IN*K*K] -> [C_IN, K*K, C_OUT] (lhsT layout per tap)
    wt = np.ascontiguousarray(
        weight.reshape(C_OUT, C_IN, K * K).transpose(1, 2, 0))
    # bias [C_OUT] -> [128, N_CHUNK]: bias2[p, c] = bias[c*128 + p]
    bias2 = np.ascontiguousarray(bias.reshape(N_CHUNK, 128).T)

    nc = _get_compiled()
    in_maps = [
        {"inp": inp[i * B_LOC:(i + 1) * B_LOC], "wt": wt, "bias2": bias2}
        for i in range(N_CORES)
    ]
    res = run_bass_kernel_spmd(nc, in_maps, list(range(N_CORES)), trace=trace)
    full = np.concatenate([res.results[i]["out"] for i in range(N_CORES)],
                          axis=0)
    return full, res


def kernel(inp, weight, bias):
    full, _ = _run(inp, weight, bias, trace=False)
    return full
